# revision 1
# baseline (speedup 1.0000x reference)
"""Bass/Trainium2 kernel for nn_BonsaiLayer (soft decision-tree layer).

Strategy (data-parallel over 8 NeuronCores, batch axis):
  - X is split host-side into fp16 planes Xh + Xl (Xl scaled by 2^11), giving
    ~22-bit effective precision for the branch-indicator matmul while keeping
    DMA traffic at 32 MB/core (same as fp32 X).
  - Planes are DMA-xbar-transposed into SBUF as X^T tiles.
  - One fused PE pass per batch tile computes [Xp | th] (th via a split
    stationary TZa/TZb at two scales, accumulated in fp32 PSUM).
  - u+/u- = sigmoid(+-2e9 * th) on ScalarE; path probabilities built level by
    level on VectorE (nodes permuted level-contiguously so every step is one
    strided instruction).
  - W/V predictors run in fp16 with batch on PSUM partitions; the probability
    multiply uses a stride-0 broadcast access pattern; node-sum is a strided
    tensor_reduce.
All shapes/strategy hardcoded for X[65536,1024], Z[64,1024], W/V[630,64], T[31,64].
"""
import sys
sys.path.insert(0, '/opt/trn_rl_repo')
import numpy as np
import concourse.bass as bass
import concourse.mybir as mybir
import concourse.tile as tile
from concourse import bacc
from concourse.bass_utils import run_bass_kernel_spmd
from concourse.masks import make_identity

F32, F16 = mybir.dt.float32, mybir.dt.float16
AF = mybir.ActivationFunctionType
OP = mybir.AluOpType

D, P, C, TOT, INT = 1024, 64, 10, 63, 31
NCORES = 8
B = 65536
BC = B // NCORES          # 8192 batch per core
NBT = BC // 512           # 16 batch tiles of 512
GRP = 4                   # batch tiles per staging group
S = 2048.0                # 2^11
O_L = [0, 1, 3, 7, 15, 31]

_ordl = [[0]]
for _ in range(5):
    _ordl.append([2 * n + 1 for n in _ordl[-1]] + [2 * n + 2 for n in _ordl[-1]])
ORDINT = _ordl[0] + _ordl[1] + _ordl[2] + _ordl[3] + _ordl[4]
PERM = ORDINT + _ordl[5]

_nc_cache = None
_last_in_maps = None


def _build_nc(reps=1, loop_reps=None):
    nc = bacc.Bacc(None, target_bir_lowering=False)
    xh_d = nc.dram_tensor("xh", [8, BC, 128], F16, kind="ExternalInput")
    xl_d = nc.dram_tensor("xl", [8, BC, 128], F16, kind="ExternalInput")
    l1_d = nc.dram_tensor("l1", [8, 128, 128], F16, kind="ExternalInput")
    l2_d = nc.dram_tensor("l2", [8, 128, 128], F16, kind="ExternalInput")
    wv_d = nc.dram_tensor("wv", [64, 1260], F16, kind="ExternalInput")
    out_d = nc.dram_tensor("out", [BC, C], F32, kind="ExternalOutput")

    with tile.TileContext(nc) as tc:
        with tc.tile_pool(name="cst", bufs=1) as cst, \
             tc.tile_pool(name="stage", bufs=2) as stage, \
             tc.tile_pool(name="work", bufs=4) as work, \
             tc.tile_pool(name="work3", bufs=5) as work3, \
             tc.tile_pool(name="mps", bufs=3, space="PSUM") as mps, \
             tc.tile_pool(name="tps", bufs=1, space="PSUM") as tps, \
             tc.tile_pool(name="wps", bufs=1, space="PSUM") as wps:

            l1_sb = cst.tile([128, 8 * 128], F16)
            l2_sb = cst.tile([128, 8 * 128], F16)
            for k in range(8):
                nc.gpsimd.dma_start(l1_sb[:, k * 128:(k + 1) * 128], l1_d[k])
                nc.gpsimd.dma_start(l2_sb[:, k * 128:(k + 1) * 128], l2_d[k])
            wv_sb = cst.tile([64, 1260], F16)
            nc.gpsimd.dma_start(wv_sb[:], wv_d[:, :])
            ident = cst.tile([INT, INT], F32)
            make_identity(nc, ident[:])
            score_sb = cst.tile([128, NBT * 4 * C], F32)

            import contextlib
            loop_ctx = tc.For_i(0, loop_reps, 1, hint_engines=tuple(nc.engines)) \
                if loop_reps else contextlib.nullcontext()
            with loop_ctx:
             for rep in range(reps):
              for g in range(NBT // GRP):
                  sh, sl = [], []
                  r0 = g * GRP * 512
                  for k in range(8):
                      th_t = stage.tile([128, GRP * 512], F16, tag=f"sh{k}")
                      nc.sync.dma_start_transpose(th_t[:], xh_d[k, r0:r0 + GRP * 512, :])
                      sh.append(th_t)
                  for k in range(8):
                      tl_t = stage.tile([128, GRP * 512], F16, tag=f"sl{k}")
                      nc.sync.dma_start_transpose(tl_t[:], xl_d[k, r0:r0 + GRP * 512, :])
                      sl.append(tl_t)

                  for bt in range(GRP):
                      t0 = g * GRP + bt
                      bs = bt * 512
                      psm = mps.tile([128, 512], F32)
                      for k in range(8):
                          nc.tensor.matmul(psm[:], l1_sb[:, k * 128:(k + 1) * 128],
                                           sh[k][:, bs:bs + 512],
                                           start=(k == 0), stop=False)
                      for k in range(8):
                          nc.tensor.matmul(psm[:], l2_sb[:, k * 128:(k + 1) * 128],
                                           sl[k][:, bs:bs + 512],
                                           start=False, stop=(k == 7))

                      xph2 = work3.tile([64, 512], F16)
                      nc.scalar.copy(xph2[:], psm[0:64, :])
                      th_b = work.tile([INT, 512], F32)
                      nc.scalar.activation(th_b[:], psm[96:127, :], AF.Copy, scale=1.0 / S)
                      th_sb = work.tile([INT, 512], F32)
                      nc.vector.tensor_tensor(th_sb[:], th_b[:], psm[64:95, :], OP.add)

                      thT = tps.tile([128, 124], F32)
                      for j in range(4):
                          nc.tensor.transpose(thT[:, j * INT:(j + 1) * INT],
                                              th_sb[:, j * 128:(j + 1) * 128], ident[:])
                      upm = work.tile([128, 248], F16)
                      nc.scalar.activation(upm[:, 0:124], thT[:], AF.Sigmoid, scale=2e9 / S)
                      nc.scalar.activation(upm[:, 124:248], thT[:], AF.Sigmoid, scale=-2e9 / S)

                      prb = work.tile([128, 252], F16)
                      p3 = prb[:].rearrange("p (j n) -> p j n", j=4)
                      nc.vector.memset(p3[:, :, 0:1], 1.0)
                      u4 = upm[:].rearrange("p (s j n) -> p j s n", s=2, j=4)
                      for l in range(1, 6):
                          h = 2 ** (l - 1)
                          out_ap = p3[:, :, O_L[l]:O_L[l] + 2 * h].rearrange(
                              "p j (s i) -> p j s i", s=2)
                          in0 = p3[:, :, O_L[l - 1]:O_L[l - 1] + h].unsqueeze(2) \
                              .broadcast_to((128, 4, 2, h))
                          in1 = u4[:, :, :, O_L[l - 1]:O_L[l - 1] + h]
                          nc.vector.tensor_tensor(out_ap, in0, in1, OP.mult)

                      for j in range(4):
                          t = t0 * 4 + j
                          wvpW = wps.tile([128, 630], F32, tag="wpsW")
                          wvpV = wps.tile([128, 630], F32, tag="wpsV")
                          lhsT = xph2[:, j * 128:(j + 1) * 128]
                          nc.tensor.matmul(wvpV[:, 0:512], lhsT, wv_sb[:, 630:1142])
                          nc.tensor.matmul(wvpV[:, 512:630], lhsT, wv_sb[:, 1142:1260])
                          tnh = work3.tile([128, 630], F16)
                          nc.scalar.activation(tnh[:], wvpV[:], AF.Tanh,
                                               scale=1.0 / S)
                          nc.tensor.matmul(wvpW[:, 0:512], lhsT, wv_sb[:, 0:512])
                          nc.tensor.matmul(wvpW[:, 512:630], lhsT, wv_sb[:, 512:630])
                          g_t = work3.tile([128, 630], F16)
                          wx_sb = work3.tile([128, 630], F16)
                          nc.scalar.copy(wx_sb[:], wvpW[:])
                          nc.vector.tensor_tensor(g_t[:], wx_sb[:], tnh[:], OP.mult)
                          h_t = work3.tile([128, 630], F16)
                          h3 = h_t[:].rearrange("p (c q) -> p c q", c=C)
                          pb = prb[:, j * TOT:(j + 1) * TOT].unsqueeze(1) \
                              .broadcast_to((128, C, TOT))
                          nc.vector.tensor_tensor(
                              h3, g_t[:].rearrange("p (c q) -> p c q", c=C), pb, OP.mult)
                          # fold 63 nodes -> 32 at 2x rate, then 1x-reduce 32
                          f_t = work3.tile([128, C * 32], F16)
                          f3 = f_t[:].rearrange("p (c q) -> p c q", c=C)
                          nc.vector.tensor_tensor(f3[:, :, 0:31], h3[:, :, 0:31],
                                                  h3[:, :, 31:62], OP.add)
                          nc.vector.tensor_copy(f3[:, :, 31:32], h3[:, :, 62:63])
                          nc.vector.tensor_reduce(score_sb[:, t * C:(t + 1) * C], f3,
                                                  axis=mybir.AxisListType.X, op=OP.add)

            nc.sync.dma_start(out_d.rearrange("(t p) c -> p t c", p=128),
                              score_sb[:].rearrange("p (t c) -> p t c", c=C))
    nc.finalize()
    return nc


def _get_nc():
    global _nc_cache
    if _nc_cache is None:
        _nc_cache = _build_nc()
    return _nc_cache


def kernel(X, Z, W, V, T):
    X = np.ascontiguousarray(np.asarray(X, dtype=np.float32))
    Z = np.asarray(Z, dtype=np.float64)
    W = np.asarray(W, dtype=np.float64)
    V = np.asarray(V, dtype=np.float64)
    T = np.asarray(T, dtype=np.float64)

    Zs = Z / P
    TZ = T[ORDINT] @ Zs                                   # [31, D]
    TZa = (TZ * S).astype(np.float16)
    TZb = ((TZ * S - TZa.astype(np.float64)) * S).astype(np.float16)
    L1 = np.zeros((D, 128), np.float16)
    L2 = np.zeros((D, 128), np.float16)
    L1[:, 0:64] = (Zs * S).astype(np.float16).T
    L1[:, 64:95] = TZa.T
    L1[:, 96:127] = TZb.T
    L2[:, 0:64] = Zs.astype(np.float16).T
    L2[:, 64:95] = (TZa.T.astype(np.float32) / S).astype(np.float16)
    LS1 = np.ascontiguousarray(L1.reshape(8, 128, 128))
    LS2 = np.ascontiguousarray(L2.reshape(8, 128, 128))

    W3 = W.reshape(TOT, C, P)
    V3 = V.reshape(TOT, C, P)
    Wt = np.ascontiguousarray(W3[PERM].transpose(2, 1, 0)).reshape(P, C * TOT)
    Vt = np.ascontiguousarray(V3[PERM].transpose(2, 1, 0)).reshape(P, C * TOT)
    WVt = np.concatenate([Wt, Vt], axis=1).astype(np.float16)   # [64, 1260]

    Xh = X.astype(np.float16)
    Xl = ((X - Xh.astype(np.float32)) * np.float32(S)).astype(np.float16)

    in_maps = []
    for c in range(NCORES):
        sl = slice(c * BC, (c + 1) * BC)
        xh_c = np.ascontiguousarray(
            Xh[sl].reshape(BC, 8, 128).transpose(1, 0, 2))
        xl_c = np.ascontiguousarray(
            Xl[sl].reshape(BC, 8, 128).transpose(1, 0, 2))
        in_maps.append({"xh": xh_c, "xl": xl_c, "l1": LS1, "l2": LS2, "wv": WVt})

    global _last_in_maps
    _last_in_maps = in_maps
    nc = _get_nc()
    res = run_bass_kernel_spmd(nc, in_maps, core_ids=list(range(NCORES)))
    score = np.concatenate([r["out"] for r in res.results], axis=0)  # [B, C]
    return np.ascontiguousarray((score.T * np.float32(1.0 / S)).astype(np.float32))

